# revision 6
# baseline (speedup 1.0000x reference)
"""Fused additive-attention kernel for Trainium2 (8 NeuronCores, SPMD).

Computes  w = softmax_K( mask ? (Wl . tanh(vW_v^T + qW_q^T) + bl) : -1e9 )
without ever materializing the [B,N,S,K,H] joint_repr intermediate.

Sharding: data-parallel over batch B (16) across 8 cores -> 2 batches/core.
Weights replicated. Host does layout prep only (transposes / packing); all
FLOPs (matmuls, tanh, softmax) run on device.

Active-box packing: masked boxes contribute exactly 0 to the softmax, so the
host packs only the active boxes of each batch into Kpk = max_b(popcount)
slots (padded lanes get -1e9 logits via the mask trick) and scatters the
packed softmax back to K=50 positions afterwards.  All tanh/add/logit work
scales by Kpk/K.  The compiled kernel depends only on Kpk (cached; rebuilt
if an input's max active count changes) - it is correct for any box_mask.

Per-core dataflow, two phases of one hc-pair (2 x 128 h-rows) each:
  qpT     [128(h), (b, hc-in-pair, ns)] bf16 per phase   (PE psum + DVE copy)
  VP2     duplicated-pair v-proj table (b, i, hc, 2)     (bv+bq folded in)
  JT      [128, (j, strip, b, hc, ns)] bf16 = QPs + vp   one DVE
          tensor_tensor per (j, strip, b) covering the hc-pair (FD=1024,
          2x_1P mode via the step-1 pair APs; TT beats tensor_scalar's
          ~210ns fixed overhead)
  tanh in-place on JT (one big ACT op per j-group; ACT = 1 elem/cycle/lane
          is the roofline engine: ~65536 cols -> ~55 us)
  logits  psum [32+kh, 512]: rows 0:kh = strip 0, rows 32:32+kh = strip 1,
          accumulated with zero-padded Wl lhsT; the strips share lhsT and
          run as adjacent matmuls (tile_position row offset 32).
  masked softmax over packed lanes after PE-transposing logits to [ns, i].
"""

import os
import sys

import numpy as np

sys.path.insert(0, "/opt/trn_rl_repo")

import concourse.bass as bass
import concourse.mybir as mybir
from concourse import bacc, bass_utils
from concourse.tile import TileContext

# Problem shapes (hardcoded per contract -- kernel.py must be self-contained)
B, N, S, K = 16, 4, 64, 50
VD, QD, H = 1024, 768, 512
NCORES = 8
BPC = B // NCORES          # batches per core = 2
NS = BPC * N * S           # 512 rows (b, n, s) per core
HC = H // 128              # 4 h-chunks
QC = QD // 128             # 6 qd-chunks
VC = VD // 128             # 8 vd-chunks
NSB = NS // BPC            # 256 (n,s) columns per batch
PH = 2                     # phases (hc-pairs)

F32 = mybir.dt.float32
BF16 = mybir.dt.bfloat16

_CACHE = {}


def _groups(kh, first_small):
    """Split range(kh) into j-groups; small leading groups if requested."""
    sizes = []
    if first_small:
        sizes = [min(2, kh), min(4, max(0, kh - 2))]
        sizes = [s for s in sizes if s > 0]
    rem = kh - sum(sizes)
    while rem > 0:
        s = min(6, rem)
        # avoid a tiny trailing group
        if rem - s in (1, 2) and s == 6:
            s = rem - 2 if rem - 2 > 0 else rem
        sizes.append(s)
        rem -= s
    out, at = [], 0
    for s in sizes:
        out.append(list(range(at, at + s)))
        at += s
    return out


def _build_nc(kpk):
    kh = kpk // 2              # strip width (psum rows 0:kh and 32:32+kh)
    kb2 = BPC * kpk            # packed (b, i) columns per core

    nc = bacc.Bacc("TRN2", target_bir_lowering=False)

    qT_h = nc.dram_tensor("qT", [QD, NS], BF16, kind="ExternalInput")
    vT_h = nc.dram_tensor("vT", [VD, kb2], BF16, kind="ExternalInput")
    # weight slabs, pre-split by hc-pair: A = h cols 0:256, B = 256:512
    WqTA_h = nc.dram_tensor("WqTA", [QD, 256], BF16, kind="ExternalInput")
    WqTB_h = nc.dram_tensor("WqTB", [QD, 256], BF16, kind="ExternalInput")
    WvTA_h = nc.dram_tensor("WvTA", [VD, 256], BF16, kind="ExternalInput")
    WvTB_h = nc.dram_tensor("WvTB", [VD, 256], BF16, kind="ExternalInput")
    # packed [128, 12]: cols 0:4 Wl chunks, 8:12 (bv+bq) chunks
    wlb_h = nc.dram_tensor("wlb", [128, 12], F32, kind="ExternalInput")
    # zero-padded Wl variants: [128, hc*kh*kh + j*kh + c] = Wl[hc*128+p]*(c==j)
    wlz_h = nc.dram_tensor("wlz", [128, HC * kh * kh], BF16, kind="ExternalInput")
    # packed [128, 2*kb2]: cols 0:kb2 validf (b,i) replicated, then (validf-1)*1e9
    msk_h = nc.dram_tensor("msk", [128, 2 * kb2], F32, kind="ExternalInput")
    id_h = nc.dram_tensor("ident", [128, 128], F32, kind="ExternalInput")
    out_h = nc.dram_tensor("out", [NS, kpk], F32, kind="ExternalOutput")

    with TileContext(nc) as tc:
        with (
            tc.tile_pool(name="persist", bufs=1) as pp,
            tc.tile_pool(name="ppsum", bufs=1, space="PSUM") as ppsum,
            tc.tile_pool(name="smpsum", bufs=2, space="PSUM") as sps,
        ):
            # ---- DMA loads, chunked + ordered so the phase-0 projection
            # chain starts after the first (qts, wqtA) chunk ----
            vts = pp.tile([128, VC, kb2], BF16, name="vts")
            nc.sync.dma_start(
                vts[:, :, :], vT_h[:, :].rearrange("(c p) j -> p c j", p=128)
            )
            qts = pp.tile([128, QC, NS], BF16, name="qts")
            wqtA = pp.tile([128, QC, 256], BF16, name="wqtA")
            qT_r = qT_h[:, :].rearrange("(c p) j -> p c j", p=128)
            wqA_r = WqTA_h[:, :].rearrange("(c p) j -> p c j", p=128)
            for c in range(3):
                nc.sync.dma_start(
                    qts[:, 2 * c : 2 * c + 2, :], qT_r[:, 2 * c : 2 * c + 2, :]
                )
                nc.sync.dma_start(
                    wqtA[:, 2 * c : 2 * c + 2, :], wqA_r[:, 2 * c : 2 * c + 2, :]
                )
                if c == 0:
                    wvtA = pp.tile([128, VC, 256], BF16, name="wvtA")
                    nc.sync.dma_start(
                        wvtA[:, :, :],
                        WvTA_h[:, :].rearrange("(c p) j -> p c j", p=128),
                    )
            wlb = pp.tile_from(wlb_h[:, :], name="wlb")
            wlz = pp.tile_from(wlz_h[:, :], name="wlz")
            msk = pp.tile_from(msk_h[:, :], name="msk")
            ident = pp.tile_from(id_h[:, :], name="ident")
            wqtB = pp.tile([128, QC, 256], BF16, name="wqtB")
            nc.sync.dma_start(
                wqtB[:, :, :], WqTB_h[:, :].rearrange("(c p) j -> p c j", p=128)
            )
            wvtB = pp.tile([128, VC, 256], BF16, name="wvtB")
            nc.sync.dma_start(
                wvtB[:, :, :], WvTB_h[:, :].rearrange("(c p) j -> p c j", p=128)
            )

            # qpT: [128, (ph, b, d, ns)] bf16 (d = hc within pair)
            QPs = pp.tile([128, PH * 2 * NS], BF16, name="QPs")
            # duplicated-pair v-proj table: [128, (ph, b, i, d, 2)] bf16
            VP2 = pp.tile([128, PH * kb2 * 4], BF16, name="VP2")

            # logits psum: rows 0:kh <- strip 0, rows 32:32+kh <- strip 1
            ps_log = ppsum.tile([32 + kh, NS], F32, name="ps_log")

            def proj_phase(ph, wqt, wvt):
                """Compute QPs/VP2 for hc-pair ph from weight slab wqt/wvt."""
                qp_v = QPs[:, ph * 2 * NS : (ph + 1) * 2 * NS].rearrange(
                    "p (b d x) -> p b d x", b=BPC, d=2
                )
                vp_v = VP2[:, ph * 4 * kb2 : (ph + 1) * 4 * kb2].rearrange(
                    "p (b i d two) -> p b i d two", b=BPC, i=kpk, d=2
                )
                with tc.tile_pool(name=f"p1ps{ph}", bufs=2, space="PSUM") as p1ps:
                    for d in range(2):
                        hc = 2 * ph + d
                        pq = p1ps.tile([128, NS], F32, tag="pq", name="pq")
                        for qc in range(QC):
                            nc.tensor.matmul(
                                pq[:, :],
                                wqt[:, qc, d * 128 : (d + 1) * 128],
                                qts[:, qc, :],
                                start=(qc == 0),
                                stop=(qc == QC - 1),
                            )
                        pv = p1ps.tile([128, kb2], F32, tag="pv", name="pv")
                        for vc in range(VC):
                            nc.tensor.matmul(
                                pv[:, :],
                                wvt[:, vc, d * 128 : (d + 1) * 128],
                                vts[:, vc, :],
                                start=(vc == 0),
                                stop=(vc == VC - 1),
                            )
                        # QPs: plain copy (bq folded into VP2's bias)
                        nc.vector.tensor_copy(
                            qp_v[:, :, d : d + 1, :],
                            pq[:, :].rearrange("p (b one x) -> p b one x",
                                               b=BPC, one=1),
                        )
                        # VP2: pair-duplicated (b, i, d, 2) with +(bv+bq)
                        nc.vector.tensor_scalar_add(
                            vp_v[:, :, :, d : d + 1, :],
                            pv[:, :]
                            .rearrange("p (b i one two) -> p b i one two",
                                       b=BPC, one=1, two=1)
                            .broadcast_to((128, BPC, kpk, 1, 2)),
                            wlb[:, 2 * HC + hc : 2 * HC + hc + 1],
                        )

            def main_phase(ph, mp, mid_cb=None):
                """Joint tanh + logit matmuls for one hc-pair."""
                groups = _groups(kh, first_small=(ph == 0))
                qp_ph = QPs[:, ph * 2 * NS : (ph + 1) * 2 * NS]
                for g, js in enumerate(groups):
                    if g == 1 and mid_cb is not None:
                        mid_cb()
                    L = len(js)
                    # JT layout: (jj, strip, b, d, x) cols
                    JT = mp.tile([128, L * 2 * 2 * 512], BF16, tag="JT", name="JT")
                    for jj, j in enumerate(js):
                        for strip in range(2):
                            i0 = j + strip * kh
                            for b in range(BPC):
                                base = ((jj * 2 + strip) * 2 + b) * 512
                                nc.vector.tensor_add(
                                    JT[:, base : base + 512].rearrange(
                                        "p (d xh two) -> p d xh two", d=2, two=2
                                    ),
                                    qp_ph[:, b * 512 : (b + 1) * 512].rearrange(
                                        "p (d xh two) -> p d xh two", d=2, two=2
                                    ),
                                    VP2[
                                        :,
                                        ph * 4 * kb2 + (b * kpk + i0) * 4
                                        : ph * 4 * kb2 + (b * kpk + i0) * 4 + 4,
                                    ]
                                    .rearrange("p (d one two) -> p d one two",
                                               d=2, one=1)
                                    .broadcast_to((128, 2, 128, 2)),
                                )
                    # in-place tanh over the whole group
                    nc.scalar.activation(
                        JT[:, :], JT[:, :], mybir.ActivationFunctionType.Tanh
                    )
                    JTv = JT[:, :].rearrange(
                        "p (jj s b d x) -> p jj s b d x", jj=L, s=2, b=BPC, d=2
                    )
                    for jj, j in enumerate(js):
                        for d in range(2):
                            hc = 2 * ph + d
                            first = ph == 0 and g == 0 and jj == 0 and d == 0
                            last = (
                                ph == PH - 1
                                and g == len(groups) - 1
                                and jj == L - 1
                                and d == 1
                            )
                            wl_col = wlz[
                                :, hc * kh * kh + j * kh : hc * kh * kh + (j + 1) * kh
                            ]
                            nc.tensor.matmul(
                                ps_log[0:kh, :],
                                wl_col,
                                JTv[:, jj : jj + 1, 0:1, :, d : d + 1, :],
                                start=first,
                                stop=last,
                                tile_position=(0, 0),
                                skip_group_check=True,
                            )
                            nc.tensor.matmul(
                                ps_log[32 : 32 + kh, :],
                                wl_col,
                                JTv[:, jj : jj + 1, 1:2, :, d : d + 1, :],
                                start=first,
                                stop=last,
                                tile_position=(0, 32),
                                skip_group_check=True,
                            )

            def proj_b():
                with tc.high_priority():
                    proj_phase(1, wqtB, wvtB)

            proj_phase(0, wqtA, wvtA)
            with tc.tile_pool(name="main", bufs=3) as mp:
                main_phase(0, mp, mid_cb=proj_b)
                main_phase(1, mp)

            # ---- masked softmax over packed lanes ----
            LG0 = pp.tile([kh, NS], F32, name="LG0")
            LG1 = pp.tile([32 + kh, NS], F32, name="LG1")
            W_all = pp.tile([128, NS // 128, kpk], F32, name="W_all")
            nc.vector.tensor_copy(LG0[:, :], ps_log[0:kh, :])
            nc.vector.tensor_copy(LG1[32 : 32 + kh, :], ps_log[32 : 32 + kh, :])
            for nsc in range(NS // 128):
                b = nsc // (NSB // 128)
                LT = pp.tile([128, kpk], F32, name=f"LT{nsc}")
                for half in range(2):
                    ps_t = sps.tile([128, kh], F32, tag="ps_t", name="ps_t")
                    if half == 0:
                        src = LG0[0:kh, nsc * 128 : (nsc + 1) * 128]
                        idn = ident[0:kh, 0:kh]
                    else:
                        src = LG1[32 : 32 + kh, nsc * 128 : (nsc + 1) * 128]
                        idn = ident[32 : 32 + kh, 32 : 32 + kh]
                    nc.tensor.transpose(ps_t[:, :], src, idn)
                    nc.vector.tensor_copy(
                        LT[:, half * kh : (half + 1) * kh], ps_t[:, :]
                    )
                # masked = logits*validf + (validf-1)*1e9
                nc.vector.tensor_mul(
                    LT[:, :], LT[:, :], msk[:, b * kpk : (b + 1) * kpk]
                )
                nc.vector.tensor_add(
                    LT[:, :], LT[:, :], msk[:, kb2 + b * kpk : kb2 + (b + 1) * kpk]
                )
                mx = pp.tile([128, 1], F32, name=f"mx{nsc}")
                nc.vector.tensor_reduce(
                    mx[:, :], LT[:, :], axis=mybir.AxisListType.X,
                    op=mybir.AluOpType.max,
                )
                mxn = pp.tile([128, 1], F32, name=f"mxn{nsc}")
                nc.vector.tensor_scalar_mul(mxn[:, :], mx[:, :], -1.0)
                EX = pp.tile([128, kpk], F32, name=f"EX{nsc}")
                sm = pp.tile([128, 1], F32, name=f"sm{nsc}")
                nc.scalar.activation(
                    EX[:, :], LT[:, :], mybir.ActivationFunctionType.Exp,
                    bias=mxn[:, 0:1], accum_out=sm[:, 0:1],
                )
                rs = pp.tile([128, 1], F32, name=f"rs{nsc}")
                nc.vector.reciprocal(rs[:, :], sm[:, :])
                nc.vector.tensor_scalar_mul(
                    W_all[:, nsc, :], EX[:, :], rs[:, 0:1]
                )
            nc.sync.dma_start(
                out_h[:, :].rearrange("(c p) j -> p c j", p=128), W_all[:, :, :]
            )

    nc.finalize()
    return nc


def _prep_in_maps(v, q, box_mask, Wv, bv, Wq, bq, Wl, kpk, active):
    """Host-side layout prep: shard over B, pack active boxes, transpose."""
    import ml_dtypes

    kh = kpk // 2
    kb2 = BPC * kpk

    v = np.asarray(v, np.float32).reshape(B, K, VD)
    q = np.asarray(q, np.float32).reshape(B, N * S, QD)

    # packed v + validity per batch
    vp = np.zeros((B, kpk, VD), np.float32)
    valid = np.zeros((B, kpk), np.float32)
    for b in range(B):
        kb = len(active[b])
        vp[b, :kb] = v[b, active[b]]
        valid[b, :kb] = 1.0

    WqT = np.asarray(Wq, np.float32).T                                # [QD, H]
    WvT = np.asarray(Wv, np.float32).T                                # [VD, H]
    WqTA = np.ascontiguousarray(WqT[:, :256]).astype(ml_dtypes.bfloat16)
    WqTB = np.ascontiguousarray(WqT[:, 256:]).astype(ml_dtypes.bfloat16)
    WvTA = np.ascontiguousarray(WvT[:, :256]).astype(ml_dtypes.bfloat16)
    WvTB = np.ascontiguousarray(WvT[:, 256:]).astype(ml_dtypes.bfloat16)
    wlb = np.zeros((128, 12), np.float32)
    wl_chunks = np.asarray(Wl, np.float32).reshape(4, 128).T          # [128, hc]
    wlb[:, 0:4] = wl_chunks
    bvq = (np.asarray(bv, np.float32) + np.asarray(bq, np.float32))
    wlb[:, 8:12] = bvq.reshape(4, 128).T
    # zero-padded Wl variants: wlz[p, hc*kh*kh + j*kh + c] = Wl_chunk[p,hc]*(c==j)
    wlz = np.zeros((128, HC, kh, kh), np.float32)
    for j in range(kh):
        wlz[:, :, j, j] = wl_chunks
    wlz = wlz.reshape(128, HC * kh * kh).astype(ml_dtypes.bfloat16)
    ident = np.eye(128, dtype=np.float32)

    in_maps = []
    for c in range(NCORES):
        b0 = c * BPC
        qc = q[b0 : b0 + BPC]                                         # [2,256,QD]
        # device QPs layout is (b, d, xh, two): plain (b, ns) on the wire;
        # qT columns are (b, ns) -> within a phase the copy re-strides.
        qT = np.ascontiguousarray(
            qc.reshape(NS, QD).T
        ).astype(ml_dtypes.bfloat16)                                  # [QD, NS]
        vc = vp[b0 : b0 + BPC].reshape(kb2, VD)
        vT = np.ascontiguousarray(vc.T).astype(ml_dtypes.bfloat16)    # [VD, kb2]
        mf = valid[b0 : b0 + BPC].reshape(1, kb2)
        msk = np.zeros((128, 2 * kb2), np.float32)
        msk[:, :kb2] = mf
        msk[:, kb2:] = (mf - 1.0) * 1e9
        in_maps.append(
            {
                "qT": qT,
                "vT": vT,
                "WqTA": WqTA,
                "WqTB": WqTB,
                "WvTA": WvTA,
                "WvTB": WvTB,
                "wlb": wlb,
                "wlz": wlz,
                "msk": msk,
                "ident": ident,
            }
        )
    return in_maps


def kernel(v, q, box_mask, tags_attention, Wv, bv, Wq, bq, Wl, bl):
    # bl shifts all unmasked logits uniformly -> cancels in softmax.
    # tags_attention is unused by the reference module.
    bm = np.asarray(box_mask).reshape(B, K)
    active = [np.nonzero(bm[b] > 0)[0] for b in range(B)]
    kmax = max(len(a) for a in active)
    if kmax == 0:
        # every box masked in every batch: reference softmax is uniform
        return np.full((B, N, S, K), 1.0 / K, np.float32)
    kpk = max(2, kmax + (kmax & 1))       # even, >= 2

    if _CACHE.get("kpk") != kpk:
        _CACHE["nc"] = _build_nc(kpk)
        _CACHE["kpk"] = kpk
    nc = _CACHE["nc"]
    in_maps = _prep_in_maps(v, q, box_mask, Wv, bv, Wq, bq, Wl, kpk, active)
    res = bass_utils.run_bass_kernel_spmd(
        nc,
        in_maps,
        core_ids=list(range(NCORES)),
        trace=bool(os.environ.get("KERNEL_TRACE")),
        tmpdir=os.environ.get("KERNEL_TMPDIR"),
    )
    _CACHE["last_result"] = res
    w = np.zeros((B, N, S, K), np.float32)
    for c in range(NCORES):
        wp = res.results[c]["out"].reshape(BPC, N, S, kpk)
        for bi in range(BPC):
            b = c * BPC + bi
            kb = len(active[b])
            if kb == 0:
                w[b] = 1.0 / K          # all-masked row: uniform softmax
            else:
                w[b][:, :, active[b]] = wp[bi][:, :, :kb]
    return w


# revision 11
# speedup vs baseline: 1.0693x; 1.0693x over previous
"""Fused additive-attention kernel for Trainium2 (8 NeuronCores, SPMD).

Computes  w = softmax_K( mask ? (Wl . tanh(vW_v^T + qW_q^T) + bl) : -1e9 )
without ever materializing the [B,N,S,K,H] joint_repr intermediate.

Sharding: data-parallel over batch B (16) across 8 cores -> 2 batches/core.
Weights replicated. Host does layout prep only (transposes / packing); all
FLOPs (matmuls, tanh, softmax) run on device.

Active-box packing: masked boxes contribute exactly 0 to the softmax, so the
host packs only the active boxes of each batch into Kpk = max_b(popcount)
slots (padded lanes get -1e9 logits via the mask trick) and scatters the
packed softmax back to K=50 positions afterwards.  All tanh/add/logit work
scales by Kpk/K.  The compiled kernel depends only on Kpk (cached; rebuilt
if an input's max active count changes) - it is correct for any box_mask.

Per-core dataflow, two phases of one hc-pair (2 x 128 h-rows) each:
  inputs  arrive as 3 host-prepacked bundles (5 flat dma_starts total; the
          first 1536 cols carry qc-chunks 0-1 so q-proj starts ~1.5us in)
  qpT     [128, (ph, d, b, ns)] bf16          (PE psum, alternating banks)
  VP2     duplicated-pair v-proj table (b, i, d, 2), bv+bq folded in
  JT      [128, (j, strip, d, b, ns)] bf16 = QPs + vp,  one DVE
          tensor_tensor per (j, strip, b) covering the hc-pair (FD=1024,
          2x pair APs -> ~414ns measured; beats tensor_scalar's overhead)
  tanh    in-place on JT, one ACT op per j-group (ACT is the roofline
          engine at 1 elem/cycle/lane: 65536 cols -> ~55us at Kpk=32)
  logits  psum [32+kh, 512]: rows 0:kh strip 0 / 32:32+kh strip 1, shared
          zero-padded Wl lhsT, adjacent matmuls via tile_position row 32;
          rhs is contiguous thanks to the (d,b,x) JT layout.
  masked softmax over packed lanes after PE-transposing logits to [ns, i];
  output leaves in native [128, (nsc, i)] layout, host de-interleaves.
"""

import os
import sys

import numpy as np

sys.path.insert(0, "/opt/trn_rl_repo")

import concourse.bass as bass
import concourse.mybir as mybir
from concourse import bacc, bass_utils
from concourse.tile import TileContext

# Problem shapes (hardcoded per contract -- kernel.py must be self-contained)
B, N, S, K = 16, 4, 64, 50
VD, QD, H = 1024, 768, 512
NCORES = 8
BPC = B // NCORES          # batches per core = 2
NS = BPC * N * S           # 512 rows (b, n, s) per core
HC = H // 128              # 4 h-chunks
QC = QD // 128             # 6 qd-chunks
VC = VD // 128             # 8 vd-chunks
NSB = NS // BPC            # 256 (n,s) columns per batch
PH = 2                     # phases (hc-pairs)

F32 = mybir.dt.float32
BF16 = mybir.dt.bfloat16

_CACHE = {}


def _groups(kh, first_small):
    """Split range(kh) into j-groups; small leading groups if requested."""
    sizes = []
    if first_small:
        sizes = [min(2, kh), min(4, max(0, kh - 2))]
        sizes = [s for s in sizes if s > 0]
    rem = kh - sum(sizes)
    while rem > 0:
        s = min(6, rem)
        if rem - s in (1, 2) and s == 6:
            s = rem - 2 if rem - 2 > 0 else rem
        sizes.append(s)
        rem -= s
    out, at = [], 0
    for s in sizes:
        out.append(list(range(at, at + s)))
        at += s
    return out


def _bnd_layout(kpk):
    """Column layout of the bf16 input bundles (per-partition views)."""
    kb2 = BPC * kpk
    # bundle A: (wqtA_qc | qts_qc) x 6, then vts, then wvtA
    xa = QC * (256 + NS) + VC * kb2 + VC * 256
    # bundle B: wqtB | wvtB | wlz
    kh = kpk // 2
    xb = QC * 256 + VC * 256 + HC * kh * kh
    return xa, xb


def _build_nc(kpk):
    kh = kpk // 2              # strip width (psum rows 0:kh and 32:32+kh)
    kb2 = BPC * kpk            # packed (b, i) columns per core
    xa, xb = _bnd_layout(kpk)
    xf = 12 + 2 * kb2 + 128    # f32 bundle: wlb | msk | ident

    nc = bacc.Bacc("TRN2", target_bir_lowering=False)

    bndA_h = nc.dram_tensor("bndA", [128, xa], BF16, kind="ExternalInput")
    bndB_h = nc.dram_tensor("bndB", [128, xb], BF16, kind="ExternalInput")
    bndF_h = nc.dram_tensor("bndF", [128, xf], F32, kind="ExternalInput")
    out_h = nc.dram_tensor("out", [128, (NS // 128) * kpk], F32,
                           kind="ExternalOutput")

    with TileContext(nc) as tc:
        with (
            tc.tile_pool(name="persist", bufs=1) as pp,
            tc.tile_pool(name="ppsum", bufs=1, space="PSUM") as ppsum,
            tc.tile_pool(name="smpsum", bufs=2, space="PSUM") as sps,
        ):
            # ---- bundle loads: 5 flat DMAs, ramp-critical slices first ----
            bndA = pp.tile([128, xa], BF16, name="bndA")
            c2 = 2 * (256 + NS)            # qc chunks 0-1
            nc.sync.dma_start(bndA[:, 0:c2], bndA_h[:, 0:c2])
            nc.sync.dma_start(bndA[:, c2 : QC * (256 + NS)],
                              bndA_h[:, c2 : QC * (256 + NS)])
            nc.sync.dma_start(bndA[:, QC * (256 + NS) :],
                              bndA_h[:, QC * (256 + NS) :])
            bndF = pp.tile([128, xf], F32, name="bndF")
            nc.sync.dma_start(bndF[:, :], bndF_h[:, :])
            bndB = pp.tile([128, xb], BF16, name="bndB")
            nc.sync.dma_start(bndB[:, :], bndB_h[:, :])

            def wq_qc(ph, qc, d):
                base = qc * (256 + NS)
                off = d * 128
                if ph == 0:
                    return bndA[:, base + off : base + off + 128]
                return bndB[:, qc * 256 + off : qc * 256 + off + 128]

            def qts_qc(qc):
                base = qc * (256 + NS) + 256
                return bndA[:, base : base + NS]

            def vts_vc(vc):
                base = QC * (256 + NS) + vc * kb2
                return bndA[:, base : base + kb2]

            def wv_vc(ph, vc, d):
                off = vc * 256 + d * 128
                if ph == 0:
                    base = QC * (256 + NS) + VC * kb2
                    return bndA[:, base + off : base + off + 128]
                return bndB[:, QC * 256 + off : QC * 256 + off + 128]

            wlz0 = QC * 256 + VC * 256
            wlb = bndF[:, 0:12]
            msk = bndF[:, 12 : 12 + 2 * kb2]
            ident = bndF[:, 12 + 2 * kb2 :]

            # qpT: [128, (ph, d, b, ns)] bf16 (d = hc within pair)
            QPs = pp.tile([128, PH * 2 * NS], BF16, name="QPs")
            # duplicated-pair v-proj table: [128, (ph, b, i, d, 2)] bf16
            VP2 = pp.tile([128, PH * kb2 * 4], BF16, name="VP2")

            # logits psum: rows 0:kh <- strip 0, rows 32:32+kh <- strip 1
            ps_log = ppsum.tile([32 + kh, NS], F32, name="ps_log")

            def proj_phase(ph):
                """Compute QPs/VP2 for hc-pair ph (d-alternating psum banks)."""
                vp_v = VP2[:, ph * 4 * kb2 : (ph + 1) * 4 * kb2].rearrange(
                    "p (b i d two) -> p b i d two", b=BPC, i=kpk, d=2
                )
                with tc.tile_pool(name=f"p1ps{ph}", bufs=1, space="PSUM") as p1ps:
                    pq = [p1ps.tile([128, NS], F32, tag=f"pq{d}", name="pq")
                          for d in range(2)]
                    for qc in range(QC):
                        for d in range(2):
                            nc.tensor.matmul(
                                pq[d][:, :],
                                wq_qc(ph, qc, d),
                                qts_qc(qc),
                                start=(qc == 0),
                                stop=(qc == QC - 1),
                            )
                    pv = [p1ps.tile([128, kb2], F32, tag=f"pv{d}",
                                    name="pv") for d in range(2)]
                    for vc in range(VC):
                        for d in range(2):
                            nc.tensor.matmul(
                                pv[d][:, :],
                                wv_vc(ph, vc, d),
                                vts_vc(vc),
                                start=(vc == 0),
                                stop=(vc == VC - 1),
                            )
                    for d in range(2):
                        hc = 2 * ph + d
                        # QPs: plain copy (bq folded into VP2's bias)
                        nc.vector.tensor_copy(
                            QPs[:, (ph * 2 + d) * NS : (ph * 2 + d + 1) * NS],
                            pq[d][:, :],
                        )
                        # VP2: pair-duplicated (b, i, d, 2) with +(bv+bq)
                        nc.vector.tensor_scalar_add(
                            vp_v[:, :, :, d : d + 1, :],
                            pv[d][:, :]
                            .rearrange("p (b i one two) -> p b i one two",
                                       b=BPC, one=1, two=1)
                            .broadcast_to((128, BPC, kpk, 1, 2)),
                            wlb[:, 2 * HC + hc : 2 * HC + hc + 1],
                        )

            def main_phase(ph, mp, mid_cb=None):
                """Joint tanh + logit matmuls for one hc-pair."""
                groups = _groups(kh, first_small=(ph == 0))
                qp_ph = QPs[:, ph * 2 * NS : (ph + 1) * 2 * NS]
                for g, js in enumerate(groups):
                    if g == 1 and mid_cb is not None:
                        mid_cb()
                    L = len(js)
                    # JT layout: (jj, strip, d, b, x) cols
                    JT = mp.tile([128, L * 2048], BF16, tag="JT", name="JT")
                    qp_v = qp_ph.rearrange(
                        "p (d b xh two) -> p d b xh two", d=2, b=BPC, xh=128
                    )
                    for jj, j in enumerate(js):
                        for strip in range(2):
                            i0 = j + strip * kh
                            sb = (jj * 2 + strip) * 1024
                            jt_v = JT[:, sb : sb + 1024].rearrange(
                                "p (d b xh two) -> p d b xh two",
                                d=2, b=BPC, xh=128,
                            )
                            for b in range(BPC):
                                vo = ph * 4 * kb2 + (b * kpk + i0) * 4
                                nc.vector.tensor_add(
                                    jt_v[:, :, b : b + 1, :, :],
                                    qp_v[:, :, b : b + 1, :, :],
                                    VP2[:, vo : vo + 4]
                                    .rearrange(
                                        "p (d bb one two) -> p d bb one two",
                                        d=2, bb=1, one=1,
                                    )
                                    .broadcast_to((128, 2, 1, 128, 2)),
                                )
                    # in-place tanh over the whole group
                    nc.scalar.activation(
                        JT[:, :], JT[:, :], mybir.ActivationFunctionType.Tanh
                    )
                    for jj, j in enumerate(js):
                        for d in range(2):
                            hc = 2 * ph + d
                            first = ph == 0 and g == 0 and jj == 0 and d == 0
                            last = (
                                ph == PH - 1
                                and g == len(groups) - 1
                                and jj == L - 1
                                and d == 1
                            )
                            wl_col = bndB[
                                :,
                                wlz0 + hc * kh * kh + j * kh
                                : wlz0 + hc * kh * kh + (j + 1) * kh,
                            ]
                            for strip in range(2):
                                rbase = (jj * 2 + strip) * 1024 + d * NS
                                nc.tensor.matmul(
                                    ps_log[32 * strip : 32 * strip + kh, :],
                                    wl_col,
                                    JT[:, rbase : rbase + NS],
                                    start=first,
                                    stop=last,
                                    tile_position=(0, 32 * strip),
                                    skip_group_check=True,
                                )

            def proj_b():
                with tc.high_priority():
                    proj_phase(1)

            proj_phase(0)
            with tc.tile_pool(name="main", bufs=3) as mp:
                main_phase(0, mp, mid_cb=proj_b)
                main_phase(1, mp)

            # ---- masked softmax over packed lanes ----
            LG0 = pp.tile([kh, NS], F32, name="LG0")
            LG1 = pp.tile([32 + kh, NS], F32, name="LG1")
            W_all = pp.tile([128, NS // 128, kpk], F32, name="W_all")
            nc.vector.tensor_copy(LG0[:, :], ps_log[0:kh, :])
            nc.vector.tensor_copy(LG1[32 : 32 + kh, :], ps_log[32 : 32 + kh, :])
            for nsc in range(NS // 128):
                b = nsc // (NSB // 128)
                LT = pp.tile([128, kpk], F32, name=f"LT{nsc}")
                for half in range(2):
                    ps_t = sps.tile([128, kh], F32, tag="ps_t", name="ps_t")
                    if half == 0:
                        src = LG0[0:kh, nsc * 128 : (nsc + 1) * 128]
                        idn = ident[0:kh, 0:kh]
                    else:
                        src = LG1[32 : 32 + kh, nsc * 128 : (nsc + 1) * 128]
                        idn = ident[32 : 32 + kh, 32 : 32 + kh]
                    nc.tensor.transpose(ps_t[:, :], src, idn)
                    nc.vector.tensor_copy(
                        LT[:, half * kh : (half + 1) * kh], ps_t[:, :]
                    )
                # masked = logits*validf + (validf-1)*1e9
                nc.vector.tensor_mul(
                    LT[:, :], LT[:, :], msk[:, b * kpk : (b + 1) * kpk]
                )
                nc.vector.tensor_add(
                    LT[:, :], LT[:, :], msk[:, kb2 + b * kpk : kb2 + (b + 1) * kpk]
                )
                mx = pp.tile([128, 1], F32, name=f"mx{nsc}")
                nc.vector.tensor_reduce(
                    mx[:, :], LT[:, :], axis=mybir.AxisListType.X,
                    op=mybir.AluOpType.max,
                )
                mxn = pp.tile([128, 1], F32, name=f"mxn{nsc}")
                nc.vector.tensor_scalar_mul(mxn[:, :], mx[:, :], -1.0)
                EX = pp.tile([128, kpk], F32, name=f"EX{nsc}")
                sm = pp.tile([128, 1], F32, name=f"sm{nsc}")
                nc.scalar.activation(
                    EX[:, :], LT[:, :], mybir.ActivationFunctionType.Exp,
                    bias=mxn[:, 0:1], accum_out=sm[:, 0:1],
                )
                rs = pp.tile([128, 1], F32, name=f"rs{nsc}")
                nc.vector.reciprocal(rs[:, :], sm[:, :])
                nc.vector.tensor_scalar_mul(
                    W_all[:, nsc, :], EX[:, :], rs[:, 0:1]
                )
                nc.sync.dma_start(
                    out_h[:, nsc * kpk : (nsc + 1) * kpk], W_all[:, nsc, :]
                )

    nc.finalize()
    return nc


def _prep_in_maps(v, q, box_mask, Wv, bv, Wq, bq, Wl, kpk, active):
    """Host-side layout prep: shard over B, pack active boxes, bundle."""
    import ml_dtypes

    kh = kpk // 2
    kb2 = BPC * kpk
    xa, xb = _bnd_layout(kpk)

    v = np.asarray(v, np.float32).reshape(B, K, VD)
    q = np.asarray(q, np.float32).reshape(B, N * S, QD)

    # packed v + validity per batch
    vpk = np.zeros((B, kpk, VD), np.float32)
    valid = np.zeros((B, kpk), np.float32)
    for b in range(B):
        kb = len(active[b])
        vpk[b, :kb] = v[b, active[b]]
        valid[b, :kb] = 1.0

    WqT = np.asarray(Wq, np.float32).T                                # [QD, H]
    WvT = np.asarray(Wv, np.float32).T                                # [VD, H]
    wl_chunks = np.asarray(Wl, np.float32).reshape(4, 128).T          # [128, hc]
    bvq = np.asarray(bv, np.float32) + np.asarray(bq, np.float32)

    # per-partition chunked views [128, C, x]
    def chunked(mT, width):  # mT [rows=c*128, width]
        c = mT.shape[0] // 128
        return mT.reshape(c, 128, width).transpose(1, 0, 2)           # [128,c,w]

    WqA = chunked(WqT[:, :256], 256)                                  # [128,6,256]
    WqB = chunked(WqT[:, 256:], 256)
    WvA = chunked(WvT[:, :256], 256)                                  # [128,8,256]
    WvB = chunked(WvT[:, 256:], 256)

    wlz = np.zeros((128, HC, kh, kh), np.float32)
    for j in range(kh):
        wlz[:, :, j, j] = wl_chunks
    wlz = wlz.reshape(128, HC * kh * kh)

    wlb = np.zeros((128, 12), np.float32)
    wlb[:, 0:4] = wl_chunks
    wlb[:, 8:12] = bvq.reshape(4, 128).T

    bndB = np.zeros((128, xb), np.float32)
    bndB[:, : QC * 256] = WqB.reshape(128, QC * 256)
    bndB[:, QC * 256 : QC * 256 + VC * 256] = WvB.reshape(128, VC * 256)
    bndB[:, QC * 256 + VC * 256 :] = wlz
    bndB = bndB.astype(ml_dtypes.bfloat16)

    ident = np.eye(128, dtype=np.float32)

    in_maps = []
    for c in range(NCORES):
        b0 = c * BPC
        qc = q[b0 : b0 + BPC].reshape(NS, QD)
        qTc = chunked(np.ascontiguousarray(qc.T), NS)                 # [128,6,NS]
        vc = vpk[b0 : b0 + BPC].reshape(kb2, VD)
        vTc = chunked(np.ascontiguousarray(vc.T), kb2)                # [128,8,kb2]

        bndA = np.zeros((128, xa), np.float32)
        for qq in range(QC):
            base = qq * (256 + NS)
            bndA[:, base : base + 256] = WqA[:, qq]
            bndA[:, base + 256 : base + 256 + NS] = qTc[:, qq]
        vo = QC * (256 + NS)
        bndA[:, vo : vo + VC * kb2] = vTc.reshape(128, VC * kb2)
        bndA[:, vo + VC * kb2 :] = WvA.reshape(128, VC * 256)
        bndA = bndA.astype(ml_dtypes.bfloat16)

        mf = valid[b0 : b0 + BPC].reshape(1, kb2)
        bndF = np.zeros((128, 12 + 2 * kb2 + 128), np.float32)
        bndF[:, 0:12] = wlb
        bndF[:, 12 : 12 + kb2] = mf
        bndF[:, 12 + kb2 : 12 + 2 * kb2] = (mf - 1.0) * 1e9
        bndF[:, 12 + 2 * kb2 :] = ident

        in_maps.append({"bndA": bndA, "bndB": bndB, "bndF": bndF})
    return in_maps


def kernel(v, q, box_mask, tags_attention, Wv, bv, Wq, bq, Wl, bl):
    # bl shifts all unmasked logits uniformly -> cancels in softmax.
    # tags_attention is unused by the reference module.
    bm = np.asarray(box_mask).reshape(B, K)
    active = [np.nonzero(bm[b] > 0)[0] for b in range(B)]
    kmax = max(len(a) for a in active)
    if kmax == 0:
        # every box masked in every batch: reference softmax is uniform
        return np.full((B, N, S, K), 1.0 / K, np.float32)
    kpk = max(2, kmax + (kmax & 1))       # even, >= 2

    if _CACHE.get("kpk") != kpk:
        _CACHE["nc"] = _build_nc(kpk)
        _CACHE["kpk"] = kpk
    nc = _CACHE["nc"]
    in_maps = _prep_in_maps(v, q, box_mask, Wv, bv, Wq, bq, Wl, kpk, active)
    res = bass_utils.run_bass_kernel_spmd(
        nc,
        in_maps,
        core_ids=list(range(NCORES)),
        trace=bool(os.environ.get("KERNEL_TRACE")),
        tmpdir=os.environ.get("KERNEL_TMPDIR"),
    )
    _CACHE["last_result"] = res
    w = np.zeros((B, N, S, K), np.float32)
    for c in range(NCORES):
        # out [128, (nsc, i)] -> rows (nsc*128+p) = (b, n, s) order
        wo = res.results[c]["out"].reshape(128, NS // 128, kpk)
        wp = wo.transpose(1, 0, 2).reshape(BPC, N, S, kpk)
        for bi in range(BPC):
            b = c * BPC + bi
            kb = len(active[b])
            if kb == 0:
                w[b] = 1.0 / K          # all-masked row: uniform softmax
            else:
                w[b][:, :, active[b]] = wp[bi][:, :, :kb]
    return w


# revision 12
# speedup vs baseline: 1.1086x; 1.0368x over previous
"""Fused additive-attention kernel for Trainium2 (8 NeuronCores, SPMD).

Computes  w = softmax_K( mask ? (Wl . tanh(vW_v^T + qW_q^T) + bl) : -1e9 )
without ever materializing the [B,N,S,K,H] joint_repr intermediate.

Sharding: data-parallel over batch B (16) across 8 cores -> 2 batches/core.
Weights replicated. Host does layout prep only (transposes / packing); all
FLOPs (matmuls, tanh, softmax) run on device.

Active-box packing: masked boxes contribute exactly 0 to the softmax, so the
host packs only the active boxes of each batch into Kpk = max_b(popcount)
slots (padded lanes get -1e9 logits via the mask trick) and scatters the
packed softmax back to K=50 positions afterwards.  All tanh/add/logit work
scales by Kpk/K.  The compiled kernel depends only on Kpk (cached; rebuilt
if an input's max active count changes) - it is correct for any box_mask.

Per-core dataflow, two phases of one hc-pair (2 x 128 h-rows) each:
  inputs  arrive as 3 host-prepacked bundles (5 flat dma_starts total; the
          first 1536 cols carry qc-chunks 0-1 so q-proj starts ~1.5us in)
  qpT     [128, (ph, d, b, ns)] bf16          (PE psum, alternating banks)
  VP2     duplicated-pair v-proj table (b, i, d, 2), bv+bq folded in
  JT      [128, (j, strip, d, b, ns)] bf16 = QPs + vp,  one DVE
          tensor_tensor per (j, strip, b) covering the hc-pair (FD=1024,
          2x pair APs -> ~414ns measured; beats tensor_scalar's overhead)
  tanh    in-place on JT, one ACT op per j-group (ACT is the roofline
          engine at 1 elem/cycle/lane: 65536 cols -> ~55us at Kpk=32)
  logits  psum [32+kh, 512]: rows 0:kh strip 0 / 32:32+kh strip 1, shared
          zero-padded Wl lhsT, adjacent matmuls via tile_position row 32;
          rhs is contiguous thanks to the (d,b,x) JT layout.
  masked softmax over packed lanes after PE-transposing logits to [ns, i];
  output leaves in native [128, (nsc, i)] layout, host de-interleaves.
"""

import os
import sys

import numpy as np

sys.path.insert(0, "/opt/trn_rl_repo")

import concourse.bass as bass
import concourse.mybir as mybir
from concourse import bacc, bass_utils
from concourse.tile import TileContext

# Problem shapes (hardcoded per contract -- kernel.py must be self-contained)
B, N, S, K = 16, 4, 64, 50
VD, QD, H = 1024, 768, 512
NCORES = 8
BPC = B // NCORES          # batches per core = 2
NS = BPC * N * S           # 512 rows (b, n, s) per core
HC = H // 128              # 4 h-chunks
QC = QD // 128             # 6 qd-chunks
VC = VD // 128             # 8 vd-chunks
NSB = NS // BPC            # 256 (n,s) columns per batch
PH = 2                     # phases (hc-pairs)

F32 = mybir.dt.float32
BF16 = mybir.dt.bfloat16

_CACHE = {}


def _groups(kh, first_small):
    """Split range(kh) into j-groups.

    first_small (phase 0): tiny leading groups so the first tanh issues
    early and DVE gets a group ahead of ACT.  Otherwise (last phase): a
    small trailing group so the post-tanh logit-matmul tail is short.
    """
    if first_small:
        pattern = [2, 2, 2, 4]
    else:
        pattern = [6, 6]
    sizes, rem = [], kh
    for s in pattern:
        s = min(s, rem)
        if s <= 0:
            break
        sizes.append(s)
        rem -= s
    if first_small:
        if rem > 0:
            sizes.append(rem)
    else:
        while rem > 0:
            s = min(2, rem)
            sizes.append(s)
            rem -= s
    out, at = [], 0
    for s in sizes:
        out.append(list(range(at, at + s)))
        at += s
    return out


def _bnd_layout(kpk):
    """Column layout of the bf16 input bundles (per-partition views)."""
    kb2 = BPC * kpk
    # bundle A: (wqtA_qc | qts_qc) x 6, then vts, then wvtA
    xa = QC * (256 + NS) + VC * kb2 + VC * 256
    # bundle B: wqtB | wvtB | wlz
    kh = kpk // 2
    xb = QC * 256 + VC * 256 + HC * kh * kh
    return xa, xb


def _build_nc(kpk):
    kh = kpk // 2              # strip width (psum rows 0:kh and 32:32+kh)
    kb2 = BPC * kpk            # packed (b, i) columns per core
    xa, xb = _bnd_layout(kpk)
    xf = 12 + 2 * kb2 + 128    # f32 bundle: wlb | msk | ident

    nc = bacc.Bacc("TRN2", target_bir_lowering=False)

    xa1 = 2 * (256 + NS)
    xa2 = (QC - 2) * (256 + NS)
    xa3 = xa - xa1 - xa2
    bndA1_h = nc.dram_tensor("bndA1", [128, xa1], BF16, kind="ExternalInput")
    bndA2_h = nc.dram_tensor("bndA2", [128, xa2], BF16, kind="ExternalInput")
    bndA3_h = nc.dram_tensor("bndA3", [128, xa3], BF16, kind="ExternalInput")
    bndB_h = nc.dram_tensor("bndB", [128, xb], BF16, kind="ExternalInput")
    bndF_h = nc.dram_tensor("bndF", [128, xf], F32, kind="ExternalInput")
    out_h = nc.dram_tensor("out", [128, (NS // 128) * kpk], F32,
                           kind="ExternalOutput")

    with TileContext(nc) as tc:
        with (
            tc.tile_pool(name="persist", bufs=1) as pp,
            tc.tile_pool(name="ppsum", bufs=1, space="PSUM") as ppsum,
            tc.tile_pool(name="smpsum", bufs=2, space="PSUM") as sps,
        ):
            # ---- bundle loads: 5 flat DMAs, ramp-critical slices first ----
            bndA = pp.tile([128, xa], BF16, name="bndA")
            c2 = 2 * (256 + NS)            # qc chunks 0-1
            nc.sync.dma_start(bndA[:, 0:c2], bndA1_h[:, :])
            nc.sync.dma_start(bndA[:, c2 : QC * (256 + NS)], bndA2_h[:, :])
            nc.sync.dma_start(bndA[:, QC * (256 + NS) :], bndA3_h[:, :])
            bndF = pp.tile([128, xf], F32, name="bndF")
            nc.sync.dma_start(bndF[:, :], bndF_h[:, :])
            bndB = pp.tile([128, xb], BF16, name="bndB")
            nc.sync.dma_start(bndB[:, :], bndB_h[:, :])
            # PE warmup: ramp the tensor-engine p-state while DMAs land
            warm = pp.tile([128, 256], BF16, name="warm")
            nc.vector.memset(warm[:, :], 0.0)
            with tc.tile_pool(name="warmps", bufs=1, space="PSUM") as wps:
                pw = wps.tile([128, 256], F32, name="pw")
                for _ in range(12):
                    nc.tensor.matmul(pw[:, :], warm[:, 0:128], warm[:, :],
                                     start=True, stop=True)

            def wq_qc(ph, qc, d):
                base = qc * (256 + NS)
                off = d * 128
                if ph == 0:
                    return bndA[:, base + off : base + off + 128]
                return bndB[:, qc * 256 + off : qc * 256 + off + 128]

            def qts_qc(qc):
                base = qc * (256 + NS) + 256
                return bndA[:, base : base + NS]

            def vts_vc(vc):
                base = QC * (256 + NS) + vc * kb2
                return bndA[:, base : base + kb2]

            def wv_vc(ph, vc, d):
                off = vc * 256 + d * 128
                if ph == 0:
                    base = QC * (256 + NS) + VC * kb2
                    return bndA[:, base + off : base + off + 128]
                return bndB[:, QC * 256 + off : QC * 256 + off + 128]

            wlz0 = QC * 256 + VC * 256
            wlb = bndF[:, 0:12]
            msk = bndF[:, 12 : 12 + 2 * kb2]
            ident = bndF[:, 12 + 2 * kb2 :]

            # qpT: [128, (ph, d, b, ns)] bf16 (d = hc within pair)
            QPs = pp.tile([128, PH * 2 * NS], BF16, name="QPs")
            # duplicated-pair v-proj table: [128, (ph, b, i, d, 2)] bf16
            VP2 = pp.tile([128, PH * kb2 * 4], BF16, name="VP2")

            # logits psum: rows 0:kh <- strip 0, rows 32:32+kh <- strip 1
            ps_log = ppsum.tile([32 + kh, NS], F32, name="ps_log")

            def proj_phase(ph):
                """Compute QPs/VP2 for hc-pair ph (d-alternating psum banks)."""
                vp_v = VP2[:, ph * 4 * kb2 : (ph + 1) * 4 * kb2].rearrange(
                    "p (b i d two) -> p b i d two", b=BPC, i=kpk, d=2
                )
                with tc.tile_pool(name=f"p1ps{ph}", bufs=1, space="PSUM") as p1ps:
                    pq = [p1ps.tile([128, NS], F32, tag=f"pq{d}", name="pq")
                          for d in range(2)]
                    for qc in range(QC):
                        for d in range(2):
                            nc.tensor.matmul(
                                pq[d][:, :],
                                wq_qc(ph, qc, d),
                                qts_qc(qc),
                                start=(qc == 0),
                                stop=(qc == QC - 1),
                            )
                    pv = [p1ps.tile([128, kb2], F32, tag=f"pv{d}",
                                    name="pv") for d in range(2)]
                    for vc in range(VC):
                        for d in range(2):
                            nc.tensor.matmul(
                                pv[d][:, :],
                                wv_vc(ph, vc, d),
                                vts_vc(vc),
                                start=(vc == 0),
                                stop=(vc == VC - 1),
                            )
                    for d in range(2):
                        hc = 2 * ph + d
                        # QPs: plain copy (bq folded into VP2's bias)
                        nc.vector.tensor_copy(
                            QPs[:, (ph * 2 + d) * NS : (ph * 2 + d + 1) * NS],
                            pq[d][:, :],
                        )
                        # VP2: pair-duplicated (b, i, d, 2) with +(bv+bq)
                        nc.vector.tensor_scalar_add(
                            vp_v[:, :, :, d : d + 1, :],
                            pv[d][:, :]
                            .rearrange("p (b i one two) -> p b i one two",
                                       b=BPC, one=1, two=1)
                            .broadcast_to((128, BPC, kpk, 1, 2)),
                            wlb[:, 2 * HC + hc : 2 * HC + hc + 1],
                        )

            def main_phase(ph, mp, mid_cb=None):
                """Joint tanh + logit matmuls for one hc-pair."""
                groups = _groups(kh, first_small=(ph == 0))
                qp_ph = QPs[:, ph * 2 * NS : (ph + 1) * 2 * NS]
                for g, js in enumerate(groups):
                    if g == 1 and mid_cb is not None:
                        mid_cb()
                    L = len(js)
                    # JT layout: (jj, strip, d, b, x) cols
                    JT = mp.tile([128, L * 2048], BF16, tag="JT", name="JT")
                    qp_v = qp_ph.rearrange(
                        "p (d b xh two) -> p d b xh two", d=2, b=BPC, xh=128
                    )
                    for jj, j in enumerate(js):
                        for strip in range(2):
                            i0 = j + strip * kh
                            sb = (jj * 2 + strip) * 1024
                            jt_v = JT[:, sb : sb + 1024].rearrange(
                                "p (d b xh two) -> p d b xh two",
                                d=2, b=BPC, xh=128,
                            )
                            for b in range(BPC):
                                vo = ph * 4 * kb2 + (b * kpk + i0) * 4
                                nc.vector.tensor_add(
                                    jt_v[:, :, b : b + 1, :, :],
                                    qp_v[:, :, b : b + 1, :, :],
                                    VP2[:, vo : vo + 4]
                                    .rearrange(
                                        "p (d bb one two) -> p d bb one two",
                                        d=2, bb=1, one=1,
                                    )
                                    .broadcast_to((128, 2, 1, 128, 2)),
                                )
                    # in-place tanh over the whole group
                    nc.scalar.activation(
                        JT[:, :], JT[:, :], mybir.ActivationFunctionType.Tanh
                    )
                    for jj, j in enumerate(js):
                        for d in range(2):
                            hc = 2 * ph + d
                            first = ph == 0 and g == 0 and jj == 0 and d == 0
                            last = (
                                ph == PH - 1
                                and g == len(groups) - 1
                                and jj == L - 1
                                and d == 1
                            )
                            wl_col = bndB[
                                :,
                                wlz0 + hc * kh * kh + j * kh
                                : wlz0 + hc * kh * kh + (j + 1) * kh,
                            ]
                            for strip in range(2):
                                rbase = (jj * 2 + strip) * 1024 + d * NS
                                nc.tensor.matmul(
                                    ps_log[32 * strip : 32 * strip + kh, :],
                                    wl_col,
                                    JT[:, rbase : rbase + NS],
                                    start=first,
                                    stop=last,
                                    tile_position=(0, 32 * strip),
                                    skip_group_check=True,
                                )

            def proj_b():
                with tc.high_priority():
                    proj_phase(1)

            proj_phase(0)
            with tc.tile_pool(name="main", bufs=3) as mp:
                main_phase(0, mp, mid_cb=proj_b)
                main_phase(1, mp)

            # ---- masked softmax over packed lanes ----
            LG0 = pp.tile([kh, NS], F32, name="LG0")
            LG1 = pp.tile([32 + kh, NS], F32, name="LG1")
            W_all = pp.tile([128, NS // 128, kpk], F32, name="W_all")
            nc.vector.tensor_copy(LG0[:, :], ps_log[0:kh, :])
            nc.vector.tensor_copy(LG1[32 : 32 + kh, :], ps_log[32 : 32 + kh, :])
            for nsc in range(NS // 128):
                b = nsc // (NSB // 128)
                LT = pp.tile([128, kpk], F32, name=f"LT{nsc}")
                for half in range(2):
                    ps_t = sps.tile([128, kh], F32, tag="ps_t", name="ps_t")
                    if half == 0:
                        src = LG0[0:kh, nsc * 128 : (nsc + 1) * 128]
                        idn = ident[0:kh, 0:kh]
                    else:
                        src = LG1[32 : 32 + kh, nsc * 128 : (nsc + 1) * 128]
                        idn = ident[32 : 32 + kh, 32 : 32 + kh]
                    nc.tensor.transpose(ps_t[:, :], src, idn)
                    nc.vector.tensor_copy(
                        LT[:, half * kh : (half + 1) * kh], ps_t[:, :]
                    )
                # masked = logits*validf + (validf-1)*1e9
                nc.vector.tensor_mul(
                    LT[:, :], LT[:, :], msk[:, b * kpk : (b + 1) * kpk]
                )
                nc.vector.tensor_add(
                    LT[:, :], LT[:, :], msk[:, kb2 + b * kpk : kb2 + (b + 1) * kpk]
                )
                mx = pp.tile([128, 1], F32, name=f"mx{nsc}")
                nc.vector.tensor_reduce(
                    mx[:, :], LT[:, :], axis=mybir.AxisListType.X,
                    op=mybir.AluOpType.max,
                )
                mxn = pp.tile([128, 1], F32, name=f"mxn{nsc}")
                nc.vector.tensor_scalar_mul(mxn[:, :], mx[:, :], -1.0)
                EX = pp.tile([128, kpk], F32, name=f"EX{nsc}")
                sm = pp.tile([128, 1], F32, name=f"sm{nsc}")
                nc.scalar.activation(
                    EX[:, :], LT[:, :], mybir.ActivationFunctionType.Exp,
                    bias=mxn[:, 0:1], accum_out=sm[:, 0:1],
                )
                rs = pp.tile([128, 1], F32, name=f"rs{nsc}")
                nc.vector.reciprocal(rs[:, :], sm[:, :])
                nc.vector.tensor_scalar_mul(
                    W_all[:, nsc, :], EX[:, :], rs[:, 0:1]
                )
                nc.sync.dma_start(
                    out_h[:, nsc * kpk : (nsc + 1) * kpk], W_all[:, nsc, :]
                )

    nc.finalize()
    return nc


def _prep_in_maps(v, q, box_mask, Wv, bv, Wq, bq, Wl, kpk, active):
    """Host-side layout prep: shard over B, pack active boxes, bundle."""
    import ml_dtypes

    kh = kpk // 2
    kb2 = BPC * kpk
    xa, xb = _bnd_layout(kpk)

    v = np.asarray(v, np.float32).reshape(B, K, VD)
    q = np.asarray(q, np.float32).reshape(B, N * S, QD)

    # packed v + validity per batch
    vpk = np.zeros((B, kpk, VD), np.float32)
    valid = np.zeros((B, kpk), np.float32)
    for b in range(B):
        kb = len(active[b])
        vpk[b, :kb] = v[b, active[b]]
        valid[b, :kb] = 1.0

    WqT = np.asarray(Wq, np.float32).T                                # [QD, H]
    WvT = np.asarray(Wv, np.float32).T                                # [VD, H]
    wl_chunks = np.asarray(Wl, np.float32).reshape(4, 128).T          # [128, hc]
    bvq = np.asarray(bv, np.float32) + np.asarray(bq, np.float32)

    # per-partition chunked views [128, C, x]
    def chunked(mT, width):  # mT [rows=c*128, width]
        c = mT.shape[0] // 128
        return mT.reshape(c, 128, width).transpose(1, 0, 2)           # [128,c,w]

    WqA = chunked(WqT[:, :256], 256)                                  # [128,6,256]
    WqB = chunked(WqT[:, 256:], 256)
    WvA = chunked(WvT[:, :256], 256)                                  # [128,8,256]
    WvB = chunked(WvT[:, 256:], 256)

    wlz = np.zeros((128, HC, kh, kh), np.float32)
    for j in range(kh):
        wlz[:, :, j, j] = wl_chunks
    wlz = wlz.reshape(128, HC * kh * kh)

    wlb = np.zeros((128, 12), np.float32)
    wlb[:, 0:4] = wl_chunks
    wlb[:, 8:12] = bvq.reshape(4, 128).T

    bndB = np.zeros((128, xb), np.float32)
    bndB[:, : QC * 256] = WqB.reshape(128, QC * 256)
    bndB[:, QC * 256 : QC * 256 + VC * 256] = WvB.reshape(128, VC * 256)
    bndB[:, QC * 256 + VC * 256 :] = wlz
    bndB = bndB.astype(ml_dtypes.bfloat16)

    ident = np.eye(128, dtype=np.float32)

    in_maps = []
    for c in range(NCORES):
        b0 = c * BPC
        qc = q[b0 : b0 + BPC].reshape(NS, QD)
        qTc = chunked(np.ascontiguousarray(qc.T), NS)                 # [128,6,NS]
        vc = vpk[b0 : b0 + BPC].reshape(kb2, VD)
        vTc = chunked(np.ascontiguousarray(vc.T), kb2)                # [128,8,kb2]

        bndA = np.zeros((128, xa), np.float32)
        for qq in range(QC):
            base = qq * (256 + NS)
            bndA[:, base : base + 256] = WqA[:, qq]
            bndA[:, base + 256 : base + 256 + NS] = qTc[:, qq]
        vo = QC * (256 + NS)
        bndA[:, vo : vo + VC * kb2] = vTc.reshape(128, VC * kb2)
        bndA[:, vo + VC * kb2 :] = WvA.reshape(128, VC * 256)
        bndA = bndA.astype(ml_dtypes.bfloat16)
        xa1 = 2 * (256 + NS)
        xa2 = (QC - 2) * (256 + NS)

        mf = valid[b0 : b0 + BPC].reshape(1, kb2)
        bndF = np.zeros((128, 12 + 2 * kb2 + 128), np.float32)
        bndF[:, 0:12] = wlb
        bndF[:, 12 : 12 + kb2] = mf
        bndF[:, 12 + kb2 : 12 + 2 * kb2] = (mf - 1.0) * 1e9
        bndF[:, 12 + 2 * kb2 :] = ident

        in_maps.append({
            "bndA1": np.ascontiguousarray(bndA[:, :xa1]),
            "bndA2": np.ascontiguousarray(bndA[:, xa1 : xa1 + xa2]),
            "bndA3": np.ascontiguousarray(bndA[:, xa1 + xa2 :]),
            "bndB": bndB,
            "bndF": bndF,
        })
    return in_maps


def kernel(v, q, box_mask, tags_attention, Wv, bv, Wq, bq, Wl, bl):
    # bl shifts all unmasked logits uniformly -> cancels in softmax.
    # tags_attention is unused by the reference module.
    bm = np.asarray(box_mask).reshape(B, K)
    active = [np.nonzero(bm[b] > 0)[0] for b in range(B)]
    kmax = max(len(a) for a in active)
    if kmax == 0:
        # every box masked in every batch: reference softmax is uniform
        return np.full((B, N, S, K), 1.0 / K, np.float32)
    kpk = max(2, kmax + (kmax & 1))       # even, >= 2

    if _CACHE.get("kpk") != kpk:
        _CACHE["nc"] = _build_nc(kpk)
        _CACHE["kpk"] = kpk
    nc = _CACHE["nc"]
    in_maps = _prep_in_maps(v, q, box_mask, Wv, bv, Wq, bq, Wl, kpk, active)
    res = bass_utils.run_bass_kernel_spmd(
        nc,
        in_maps,
        core_ids=list(range(NCORES)),
        trace=bool(os.environ.get("KERNEL_TRACE")),
        tmpdir=os.environ.get("KERNEL_TMPDIR"),
    )
    _CACHE["last_result"] = res
    w = np.zeros((B, N, S, K), np.float32)
    for c in range(NCORES):
        # out [128, (nsc, i)] -> rows (nsc*128+p) = (b, n, s) order
        wo = res.results[c]["out"].reshape(128, NS // 128, kpk)
        wp = wo.transpose(1, 0, 2).reshape(BPC, N, S, kpk)
        for bi in range(BPC):
            b = c * BPC + bi
            kb = len(active[b])
            if kb == 0:
                w[b] = 1.0 / K          # all-masked row: uniform softmax
            else:
                w[b][:, :, active[b]] = wp[bi][:, :, :kb]
    return w
